# revision 1
# baseline (speedup 1.0000x reference)
"""BinLinear TRN2 kernel: out = x @ sign(weight).T + sign(bias).

Full shapes: x [8192, 4096] f32, weight [4096, 4096] f32, bias [4096] f32
-> out [8192, 4096] f32.

Sharding (8 NeuronCores): 2D grid, 4-way over tokens x 2-way over output
features. Each core computes out_c = x_c @ sign(w_c).T + sign(b_c) with
x_c [2048, 4096], w_c [2048, 4096], b_c [2048] -> out_c [2048, 2048].
The host only slices inputs and stitches the 4x2 output grid back together.

Per-core device program (fp16 single-pass matmul, everything on device).
HBM traffic is the bare minimum (one fp32 read of x and w, one fp32 write
of out, ~84MB/core); both operands are transposed on the fly:
  - 128-row slabs of w and x are cast fp32->fp16 directly DRAM->SBUF by
    SWDGE DMA (one serial stream, explicitly ordered: the first quarter of
    w first, then the first x tiles, then the rest of w, then x tiles
    paced behind consumption).
  - Each staged slab is XBAR dma-transposed SBUF->SBUF into resident
    wT[i-part, kt, o] tiles / xT[i-part, kt, tok] tiles.
  - Weight slabs are binarized in place by one DVE op per slab:
    (w16 > 0) - 0.5 = 0.5*sign(w) (fp16 compare handles subnormals
    exactly -- verified on HW; exact-zero weights use a fallback variant
    selected on the host).
  - PE: per [128-token, 512-feature] PSUM tile: one K=1 matmul seeds the
    bias row (ones^T @ 0.5*sign(b)), then 32 K=128 fp16 matmuls accumulate
    x @ w_sign.  Copy-back scales by 2 on DVE into fp32 output tiles.
  - Schedule: the first 4 token tiles run feature-block-major against the
    streaming weight prep; the remaining 12 run token-major with
    kt-outer/feature-inner matmuls (4 PSUM banks accumulating at once).
"""

import sys

if "/opt/trn_rl_repo" not in sys.path:
    sys.path.insert(0, "/opt/trn_rl_repo")

from contextlib import ExitStack

import numpy as np

import concourse.bass as bass
import concourse.mybir as mybir
import concourse.tile as tile
from concourse import bacc
from concourse.bass_utils import run_bass_kernel_spmd
from concourse.tile_rust import add_dep_helper

N_TOK, D_IN, D_OUT = 8192, 4096, 4096
TOK_WAYS, OUT_WAYS = 4, 2
N_CORES = TOK_WAYS * OUT_WAYS
TOK_SH = N_TOK // TOK_WAYS    # 2048 tokens per core
OUT_SH = D_OUT // OUT_WAYS    # 2048 out features per core

P = 128
KT = D_IN // P                # 32 contraction subtiles
NFREE = 512                   # PSUM free dim per matmul
NB = OUT_SH // NFREE          # 4 feature blocks
TB = TOK_SH // P              # 16 token tiles
NWS = OUT_SH // P             # 16 weight slabs

F16 = mybir.dt.float16
F32 = mybir.dt.float32


def _build(exact_sign: bool):
    """Build the per-core SPMD program."""
    nc = bacc.Bacc("TRN2", target_bir_lowering=False, debug=False,
                   num_devices=N_CORES)
    x = nc.dram_tensor("x", [TOK_SH, D_IN], F32, kind="ExternalInput")
    w = nc.dram_tensor("w", [OUT_SH, D_IN], F32, kind="ExternalInput")
    b = nc.dram_tensor("b", [1, OUT_SH], F32, kind="ExternalInput")
    out = nc.dram_tensor("out", [TOK_SH, OUT_SH], F32, kind="ExternalOutput")

    with ExitStack() as ctx:
        tc = ctx.enter_context(tile.TileContext(nc))
        dramp = ctx.enter_context(tc.tile_pool(name="dramp", bufs=1, space="DRAM"))
        wTp = ctx.enter_context(tc.tile_pool(name="wTp", bufs=NB))
        # exact-sign variant needs DVE temp tiles; shrink xT for it
        xTp = ctx.enter_context(tc.tile_pool(name="xTp", bufs=2))
        stagep = ctx.enter_context(tc.tile_pool(name="stagep", bufs=2))
        sgtmp = ctx.enter_context(tc.tile_pool(name="sgtmp", bufs=2))
        osbp = ctx.enter_context(tc.tile_pool(name="osbp", bufs=2))
        constp = ctx.enter_context(tc.tile_pool(name="constp", bufs=1))
        mmps = ctx.enter_context(tc.tile_pool(name="mmps", bufs=8, space="PSUM"))

        def sign_inplace(ap, tmp_shape, tag):
            """ap = 0.5*sign(ap) elementwise on fp16 data (in place)."""
            if exact_sign:
                t1 = sgtmp.tile(tmp_shape, F16, tag=tag, name=f"{tag}_t")
                nc.vector.tensor_scalar(t1[:], ap, 0.0, None, mybir.AluOpType.is_lt)
                nc.vector.tensor_scalar(ap, ap, 0.0, None, mybir.AluOpType.is_gt)
                nc.vector.tensor_tensor(ap, ap, t1[:], mybir.AluOpType.subtract)
                nc.vector.tensor_scalar(ap, ap, 0.5, None, mybir.AluOpType.mult)
            else:
                nc.vector.tensor_scalar(
                    ap, ap, 0.0, 0.5,
                    mybir.AluOpType.is_gt, mybir.AluOpType.subtract,
                )

        # ---- constants
        ones = constp.tile([1, P], F16)
        nc.gpsimd.memset(ones[:], 1.0)

        # ---- SWDGE cast stream (DRAM->SBUF fp32->fp16), explicitly
        # ordered via nosync deps so weight prep is never starved.
        last_swdge = [None]

        def swdge_cast(dst_ap, src_ap, pace_dep=None):
            inst = nc.gpsimd.dma_start(dst_ap, src_ap)
            if last_swdge[0] is not None:
                add_dep_helper(inst.ins, last_swdge[0].ins, sync=False,
                               reason="SWDGE cast order")
            if pace_dep is not None:
                add_dep_helper(inst.ins, pace_dep.ins, sync=True,
                               reason="pace x cast behind xT consumption")
            last_swdge[0] = inst
            return inst

        wT = [wTp.tile([P, KT, NFREE], F16, tag="wT", name=f"wT{i}")
              for i in range(NB)]

        DSL = 1 if exact_sign else 2   # weight slabs per SWDGE cast op

        def w_slab(j2):
            """Cast DSL 128-row weight slabs in one SWDGE op, then XBAR+sign
            each slab."""
            st = stagep.tile([P, DSL, D_IN], F16, tag="stage", name=f"wst{j2}")
            src_ap = w[j2 * DSL * P : (j2 + 1) * DSL * P, :]
            if DSL > 1:
                src_ap = src_ap.rearrange("(a p) d -> p a d", p=P)
                swdge_cast(st[:], src_ap)
            else:
                swdge_cast(st[:, 0, :], src_ap)
            for a in range(DSL):
                j = j2 * DSL + a
                nb, jj = j // NB, j % NB
                dst = wT[nb][:, :, jj * P : (jj + 1) * P]
                nc.sync.dma_start_transpose(dst, st[:, a, :])
                sign_inplace(dst, [P, KT, P], "wsg")

        # ---- x: SWDGE cast fp32 -> fp16 into 256-token DRAM chunks
        # (DRAM->DRAM streams faster per byte than DRAM->SBUF casts), then
        # XBAR transpose DRAM->SBUF per use.
        SC = 2 * P               # 256-token super-chunks
        NSC = TOK_SH // SC       # 8
        x16 = [None] * NSC
        cast_issued = [False] * NSC

        def issue_x_cast(s, dep=None):
            if cast_issued[s]:
                return
            cast_issued[s] = True
            ch = dramp.tile([SC, D_IN], F16, tag="x16", name=f"x16_{s}", bufs=NSC)
            swdge_cast(ch[:], x[s * SC : (s + 1) * SC, :], pace_dep=dep)
            x16[s] = ch

        def load_xT(s):
            xT = xTp.tile([P, KT, SC], F16, tag="xT", name=f"xT_{s}")
            inst = nc.sync.dma_start_transpose(xT[:], x16[s][:])
            return xT, inst

        # Cast order: w slabs 0..3 (wT[0]), x chunks 0,1, w slabs 4..15,
        # then x chunks 2,3 paced on the first transposes.  SP-ring order:
        # wT XBARs 0..3, xT XBARs 0,1, wT XBARs 4..15.
        # ---- bias row: brow = 0.5*sign(b) as fp16 [1, OUT_SH]
        # (bf32 is transient; it shares the staging slots)
        bf32 = stagep.tile([1, OUT_SH], F32, tag="stage", name="bf32")
        nc.scalar.dma_start(bf32[:], b[:])
        brow = constp.tile([1, OUT_SH], F16)
        if exact_sign:
            bt = constp.tile([1, OUT_SH], F16)
            nc.vector.tensor_scalar(bt[:], bf32[:], 0.0, None, mybir.AluOpType.is_lt)
            nc.vector.tensor_scalar(brow[:], bf32[:], 0.0, None, mybir.AluOpType.is_gt)
            nc.vector.tensor_tensor(brow[:], brow[:], bt[:], mybir.AluOpType.subtract)
            nc.vector.tensor_scalar(brow[:], brow[:], 0.5, None, mybir.AluOpType.mult)
        else:
            nc.vector.tensor_scalar(
                brow[:], bf32[:], 0.0, 0.5,
                mybir.AluOpType.is_gt, mybir.AluOpType.subtract,
            )

        NW2 = NWS // DSL
        w_slab(0)                      # slabs 0..DSL-1
        issue_x_cast(0)
        w_slab(1)                      # completes wT[0] when DSL=2
        issue_x_cast(1)
        xT0, xT0_inst = load_xT(0)
        xT1, xT1_inst = load_xT(1)
        for j2 in range(2, NW2):
            w_slab(j2)
        issue_x_cast(2, dep=xT0_inst)
        issue_x_cast(3, dep=xT1_inst)

        def bias_mm(psum, nb):
            nc.tensor.matmul(
                psum[:], ones[:], brow[:, nb * NFREE : (nb + 1) * NFREE],
                start=True, stop=False,
            )

        def copy_out(psum, row0, nb):
            osb = osbp.tile([P, NFREE], F32, tag="osb", name="osb")
            nc.vector.tensor_scalar(osb[:], psum[:], 2.0, None, mybir.AluOpType.mult)
            nc.scalar.dma_start(
                out[row0 : row0 + P, nb * NFREE : (nb + 1) * NFREE], osb[:])

        def block(xT, s, half, nb):
            psum = mmps.tile([P, NFREE], F32, tag="mm", name="psum")
            bias_mm(psum, nb)
            for kt in range(KT):
                nc.tensor.matmul(
                    psum[:], xT[:, kt, half * P : (half + 1) * P],
                    wT[nb][:, kt, :],
                    start=False, stop=(kt == KT - 1),
                )
            copy_out(psum, s * SC + half * P, nb)

        # ---- Phase 1: chunks 0-1 resident, feature-block-major -- matmuls
        # start on wT[0] while the weight stream is still arriving.
        for nb in range(NB):
            for s in range(2):
                for half in range(2):
                    block([xT0, xT1][s], s, half, nb)

        # ---- Phase 1.5: chunks 2-3 in two feature-pair passes (reloading
        # xT per pass) -- still only needs the first half/full of wT late.
        for nbp in range(2):
            for s in (2, 3):
                xT, inst = load_xT(s)
                if nbp == 0 and s == 2:
                    issue_x_cast(4, dep=inst)
                if nbp == 0 and s == 3:
                    issue_x_cast(5, dep=inst)
                for half in range(2):
                    psums = [mmps.tile([P, NFREE], F32, tag="mm", name=f"ps{i}")
                             for i in range(2)]
                    for i, nb in enumerate((2 * nbp, 2 * nbp + 1)):
                        bias_mm(psums[i], nb)
                    for kt in range(KT):
                        lhsT = xT[:, kt, half * P : (half + 1) * P]
                        for i, nb in enumerate((2 * nbp, 2 * nbp + 1)):
                            nc.tensor.matmul(
                                psums[i][:], lhsT, wT[nb][:, kt, :],
                                start=False, stop=(kt == KT - 1),
                            )
                    for i, nb in enumerate((2 * nbp, 2 * nbp + 1)):
                        copy_out(psums[i], s * SC + half * P, nb)

        # ---- Phase 2: chunks 4-7, token-major; kt-outer/feature-inner so
        # four PSUM banks accumulate per stationary load.
        for s in range(4, NSC):
            xT, inst = load_xT(s)
            if s + 2 < NSC:
                issue_x_cast(s + 2, dep=inst)
            for half in range(2):
                psums = [mmps.tile([P, NFREE], F32, tag="mm", name=f"psum{i}")
                         for i in range(NB)]
                for nb in range(NB):
                    bias_mm(psums[nb], nb)
                for kt in range(KT):
                    lhsT = xT[:, kt, half * P : (half + 1) * P]
                    for nb in range(NB):
                        nc.tensor.matmul(
                            psums[nb][:], lhsT, wT[nb][:, kt, :],
                            start=False, stop=(kt == KT - 1),
                        )
                for nb in range(NB):
                    copy_out(psums[nb], s * SC + half * P, nb)

    nc.finalize()
    return nc


_cache = {}


def _get_nc(exact_sign: bool):
    if exact_sign not in _cache:
        _cache[exact_sign] = _build(exact_sign)
    return _cache[exact_sign]


def kernel(x: np.ndarray, weight: np.ndarray, bias: np.ndarray) -> np.ndarray:
    x = np.ascontiguousarray(np.asarray(x, dtype=np.float32))
    weight = np.ascontiguousarray(np.asarray(weight, dtype=np.float32))
    bias = np.ascontiguousarray(np.asarray(bias, dtype=np.float32))
    assert x.shape == (N_TOK, D_IN) and weight.shape == (D_OUT, D_IN)

    # (w > 0) - 0.5 equals 0.5*sign(w) only when no exact zeros exist;
    # fall back to the exact 3-op sign variant otherwise.
    exact_sign = bool((weight == 0.0).any() or (bias == 0.0).any())
    nc = _get_nc(exact_sign)

    in_maps = []
    for tg in range(TOK_WAYS):
        for og in range(OUT_WAYS):
            in_maps.append({
                "x": np.ascontiguousarray(x[tg * TOK_SH : (tg + 1) * TOK_SH, :]),
                "w": np.ascontiguousarray(weight[og * OUT_SH : (og + 1) * OUT_SH, :]),
                "b": np.ascontiguousarray(bias[og * OUT_SH : (og + 1) * OUT_SH].reshape(1, OUT_SH)),
            })

    res = run_bass_kernel_spmd(nc, in_maps, list(range(N_CORES)))

    out = np.empty((N_TOK, D_OUT), dtype=np.float32)
    c = 0
    for tg in range(TOK_WAYS):
        for og in range(OUT_WAYS):
            out[tg * TOK_SH : (tg + 1) * TOK_SH, og * OUT_SH : (og + 1) * OUT_SH] = \
                res.results[c]["out"]
            c += 1
    return out



# revision 5
# speedup vs baseline: 1.0042x; 1.0042x over previous
"""BinLinear TRN2 kernel: out = x @ sign(weight).T + sign(bias).

Full shapes: x [8192, 4096] f32, weight [4096, 4096] f32, bias [4096] f32
-> out [8192, 4096] f32.

Sharding (8 NeuronCores): 2D grid, 4-way over tokens x 2-way over output
features. Each core computes out_c = x_c @ sign(w_c).T + sign(b_c) with
x_c [2048, 4096], w_c [2048, 4096], b_c [2048] -> out_c [2048, 2048].
The host only slices inputs and stitches the 4x2 output grid back together.

Per-core device program (fp16 single-pass matmul, everything on device):
  - One SWDGE cast chain streams 128-row slabs of w and x fp32->fp16
    DRAM->SBUF, interleaved x0,w0,w1,x1,w2,w3,... so the PE's available
    work frontier (arrived w-pairs x arrived x-slabs) grows quadratically
    while the stream is linear -- no mid-kernel starvation.
  - Each staged slab is XBAR dma-transposed SBUF->SBUF into resident
    wT pair-tiles [128, 32kt, 256feat] (8 of them) / a ring of xT slab
    tiles [128, 32kt, 128tok].
  - Weight slabs are binarized in place by one fused DVE op:
    (w16 > 0) - 0.5 = 0.5*sign(w) (exact-zero weights use a 3-op exact
    variant selected on the host; never needed for randn inputs).
  - PE: uniform [128-token, 256-feature] PSUM cells, 32 K=128 fp16
    matmuls each.  N=256 runs at full PE rate (107ns matmul still hides
    the 97ns LDWEIGHTS) and lets a cell depend on a single w pair-tile,
    so compute starts ~25us in on partially-arrived weights.
  - No bias matmuls: copy-back is one DVE scalar_tensor_tensor
    osb = psum*2 + sign(b) (bias row partition-broadcast), then DMA out.
"""

import sys

if "/opt/trn_rl_repo" not in sys.path:
    sys.path.insert(0, "/opt/trn_rl_repo")

from contextlib import ExitStack

import numpy as np

import concourse.bass as bass
import concourse.mybir as mybir
import concourse.tile as tile
from concourse import bacc
from concourse.bass_utils import run_bass_kernel_spmd
from concourse.tile_rust import add_dep_helper

N_TOK, D_IN, D_OUT = 8192, 4096, 4096
TOK_WAYS, OUT_WAYS = 4, 2
N_CORES = TOK_WAYS * OUT_WAYS
TOK_SH = N_TOK // TOK_WAYS    # 2048 tokens per core
OUT_SH = D_OUT // OUT_WAYS    # 2048 out features per core

P = 128
KT = D_IN // P                # 32 contraction subtiles
NFREE = 256                   # PSUM free dim per matmul (one w pair-tile)
NSL = TOK_SH // P             # 16 token slabs
NWS = OUT_SH // P             # 16 weight slabs
NPAIR = NWS // 2              # 8 weight pair-tiles

F16 = mybir.dt.float16
F32 = mybir.dt.float32


def _build(exact_sign: bool):
    """Build the per-core SPMD program."""
    nc = bacc.Bacc("TRN2", target_bir_lowering=False, debug=False,
                   num_devices=N_CORES)
    x = nc.dram_tensor("x", [TOK_SH, D_IN], F32, kind="ExternalInput")
    w = nc.dram_tensor("w", [OUT_SH, D_IN], F32, kind="ExternalInput")
    b = nc.dram_tensor("b", [1, OUT_SH], F32, kind="ExternalInput")
    out = nc.dram_tensor("out", [TOK_SH, OUT_SH], F32, kind="ExternalOutput")

    PRO_S = 4 if exact_sign else 5   # x slabs resident during the prologue
    RING = PRO_S                     # xT ring size

    with ExitStack() as ctx:
        tc = ctx.enter_context(tile.TileContext(nc))
        wTp = ctx.enter_context(tc.tile_pool(name="wTp", bufs=NPAIR))
        xTp = ctx.enter_context(tc.tile_pool(name="xTp", bufs=RING))
        stagep = ctx.enter_context(tc.tile_pool(name="stagep", bufs=2))
        sgtmp = ctx.enter_context(tc.tile_pool(name="sgtmp", bufs=2))
        osbp = ctx.enter_context(tc.tile_pool(name="osbp", bufs=3))
        constp = ctx.enter_context(tc.tile_pool(name="constp", bufs=1))
        mmps = ctx.enter_context(tc.tile_pool(name="mmps", bufs=8, space="PSUM"))

        def sign_half_inplace(ap, tmp_shape, tag):
            """ap = 0.5*sign(ap) elementwise on fp16 data (in place)."""
            if exact_sign:
                t1 = sgtmp.tile(tmp_shape, F16, tag=tag, name=f"{tag}_t")
                nc.vector.tensor_scalar(t1[:], ap, 0.0, None, mybir.AluOpType.is_lt)
                nc.vector.tensor_scalar(ap, ap, 0.0, None, mybir.AluOpType.is_gt)
                nc.vector.tensor_tensor(ap, ap, t1[:], mybir.AluOpType.subtract)
                nc.vector.tensor_scalar(ap, ap, 0.5, None, mybir.AluOpType.mult)
            else:
                nc.vector.tensor_scalar(
                    ap, ap, 0.0, 0.5,
                    mybir.AluOpType.is_gt, mybir.AluOpType.subtract,
                )

        # ---- SWDGE cast chain (DRAM fp32 -> SBUF fp16), nosync-ordered so
        # slabs complete in stream order without hard pacing stalls.
        last_swdge = [None]

        def swdge_cast(dst_ap, src_ap):
            inst = nc.gpsimd.dma_start(dst_ap, src_ap)
            if last_swdge[0] is not None:
                add_dep_helper(inst.ins, last_swdge[0].ins, sync=False,
                               reason="SWDGE cast order")
            last_swdge[0] = inst
            return inst

        # ---- bias: brow = sign(b) as fp16, partition-broadcast to all 128
        # rows once via a 0-stride DMA read so copy-back is a plain
        # tensor_tensor add.
        brow = constp.tile([P, OUT_SH], F16)
        swdge_cast(brow[:], b[0:1, :].partition_broadcast(P))
        if exact_sign:
            bt = constp.tile([P, OUT_SH], F16)
            nc.vector.tensor_scalar(bt[:], brow[:], 0.0, None, mybir.AluOpType.is_lt)
            nc.vector.tensor_scalar(brow[:], brow[:], 0.0, None, mybir.AluOpType.is_gt)
            nc.vector.tensor_tensor(brow[:], brow[:], bt[:], mybir.AluOpType.subtract)
        else:
            nc.vector.tensor_scalar(
                brow[:], brow[:], 0.0, 0.5,
                mybir.AluOpType.is_gt, mybir.AluOpType.subtract,
            )
            nc.vector.tensor_scalar(brow[:], brow[:], 2.0, None,
                                    mybir.AluOpType.mult)

        # ---- resident weight pair-tiles and the xT ring
        wT = [wTp.tile([P, KT, NFREE], F16, tag="wT", name=f"wT{q}")
              for q in range(NPAIR)]
        xT = [None] * NSL

        def w_slab(j):
            """Stream weight slab j: cast, transpose, binarize."""
            st = stagep.tile([P, D_IN], F16, tag="stage", name=f"wst{j}")
            swdge_cast(st[:], w[j * P:(j + 1) * P, :])
            q, jj = j // 2, j % 2
            dst = wT[q][:, :, jj * P:(jj + 1) * P]
            nc.sync.dma_start_transpose(dst, st[:])
            sign_half_inplace(dst, [P, KT, P], "wsg")

        def x_slab(s):
            """Stream token slab s: cast, transpose into ring slot s%RING."""
            st = stagep.tile([P, D_IN], F16, tag="stage", name=f"xst{s}")
            swdge_cast(st[:], x[s * P:(s + 1) * P, :])
            xT[s] = xTp.tile([P, KT, P], F16, tag="xT", name=f"xT{s}")
            nc.sync.dma_start_transpose(xT[s][:], st[:])

        def cell(q, s):
            """One [128-token, 256-feature] output cell: 32 matmuls, fused
            scale+bias copy-back, DMA out."""
            psum = mmps.tile([P, NFREE], F32, tag="mm", name="psum")
            for kt in range(KT):
                nc.tensor.matmul(
                    psum[:], xT[s][:, kt, :], wT[q][:, kt, :],
                    start=(kt == 0), stop=(kt == KT - 1),
                )
            osb = osbp.tile([P, NFREE], F32, tag="osb", name="osb")
            nc.vector.scalar_tensor_tensor(
                osb[:], psum[:], 2.0,
                brow[:, q * NFREE:(q + 1) * NFREE],
                mybir.AluOpType.mult, mybir.AluOpType.add,
            )
            nc.scalar.dma_start(
                out[s * P:(s + 1) * P, q * NFREE:(q + 1) * NFREE], osb[:])

        # ---- stream + prologue wavefront over (pair r, slab s) cells.
        # Stream order: x0,w0,w1 | x1,w2,w3 | ... | x{PRO_S-1},w..,.. | rest
        # of w pairs | x{PRO_S}..x15.  Cell (q, s) is emitted at round
        # max(q, s) so it only needs arrived data.
        x_slab(0)
        for r in range(NPAIR):
            w_slab(2 * r)
            w_slab(2 * r + 1)
            if r + 1 < PRO_S:
                x_slab(r + 1)
            for s in range(min(r + 1, PRO_S)):
                cell(r, s)
            if r < PRO_S - 1:
                for q in range(r + 1):
                    cell(q, r + 1)

        # ---- bulk: remaining token slabs, slab-major (wT fully resident).
        for s in range(PRO_S, NSL):
            x_slab(s)
            for q in range(NPAIR):
                cell(q, s)

    nc.finalize()
    return nc


_cache = {}


def _get_nc(exact_sign: bool):
    if exact_sign not in _cache:
        _cache[exact_sign] = _build(exact_sign)
    return _cache[exact_sign]


def kernel(x: np.ndarray, weight: np.ndarray, bias: np.ndarray) -> np.ndarray:
    x = np.ascontiguousarray(np.asarray(x, dtype=np.float32))
    weight = np.ascontiguousarray(np.asarray(weight, dtype=np.float32))
    bias = np.ascontiguousarray(np.asarray(bias, dtype=np.float32))
    assert x.shape == (N_TOK, D_IN) and weight.shape == (D_OUT, D_IN)

    # (w > 0) - 0.5 equals 0.5*sign(w) only when no exact zeros exist;
    # fall back to the exact 3-op sign variant otherwise.
    exact_sign = bool((weight == 0.0).any() or (bias == 0.0).any())
    nc = _get_nc(exact_sign)

    in_maps = []
    for tg in range(TOK_WAYS):
        for og in range(OUT_WAYS):
            in_maps.append({
                "x": np.ascontiguousarray(x[tg * TOK_SH:(tg + 1) * TOK_SH, :]),
                "w": np.ascontiguousarray(weight[og * OUT_SH:(og + 1) * OUT_SH, :]),
                "b": np.ascontiguousarray(
                    bias[og * OUT_SH:(og + 1) * OUT_SH].reshape(1, OUT_SH)),
            })

    res = run_bass_kernel_spmd(nc, in_maps, list(range(N_CORES)))

    out = np.empty((N_TOK, D_OUT), dtype=np.float32)
    c = 0
    for tg in range(TOK_WAYS):
        for og in range(OUT_WAYS):
            out[tg * TOK_SH:(tg + 1) * TOK_SH, og * OUT_SH:(og + 1) * OUT_SH] = \
                res.results[c]["out"]
            c += 1
    return out
